# revision 23
# baseline (speedup 1.0000x reference)
"""Trainium2 Bass kernel for nn_CodeSynthesisModel (gnn_message_passing).

Data-parallel over 8 NeuronCores: B=64 sharded 8 ways (8 batches/core),
weights replicated. All O(B*N) compute runs on device via a Bass/Tile
kernel (one NEFF, SPMD on cores 0-7 through run_bass_kernel_spmd).

Math (exact, validated in numpy vs the reference):
  - The attention scorer is affine: w = Wa1@Wa2 collapses it to
      att[n] = q[n] + k_b,   q[n] = sum_j g_j[t_j[n]],
    where g_j are 200-entry scalar lookup tables (weight-projected pe /
    embedding / per-batch lstm rows). All per-node gathers (4 cols x 4096
    nodes x 8 batches) are ONE gpsimd ap_gather from per-partition
    tables with per-Q7-core index streams.
  - att_sum = sum_n att[n] * node_vec[n] decomposes into weighted
    histograms. With t = 16*lo + hi the histogram H2d[hi,lo] is a PE
    matmul (att*onehot_hi)^T @ onehot_lo per (batch, 128-node chunk),
    accumulated in PSUM; one-hots built on DVE in 2 big instructions.
  - att_sum block j = sum_lo H2d[:,lo]^T @ T_j[16*lo:16*lo+16, :] -- 13
    small PE matmuls per j. `last` (node 4095) uses the same projection
    path with a rank-1 one-hot outer product.
  - The tail MLPs are tiny PE matmuls with bias rows folded in; the
    hidden vector is assembled in a 32-aligned row layout (DVE partition
    bases must be multiples of 32) and W1 is host-permuted to match.

Layout: col = ch*8 + b (ch = 128-node chunk, b = local batch); node
n = ch*128 + p. One-hot j-blocks in order JS = [3,0,1,2] so the lstm
block sits at rows 0:16 and pe/pe/emb at rows 32/64/96.
"""

import numpy as np
import ml_dtypes

B, N, V = 64, 4096, 200
NCORES, BPC = 8, 8
NCH = 32
NCOL = 256
NI = 16400                  # gather indices (16384 + 4 + 12 pad)
GT = 2224                   # gather table entries
JS = [3, 0, 1, 2]
JHI = 4112                  # per-j extent in oh_hi (257*16, one pad col)

_CACHE = {}


def _make_pe():
    pos = np.arange(V, dtype=np.float32)[:, None]
    div = np.exp(np.arange(0, 8, 2, dtype=np.float32) * (-np.log(10000.0) / 8))
    pe = np.zeros((V, 8), dtype=np.float32)
    pe[:, 0::2] = np.sin(pos * div)
    pe[:, 1::2] = np.cos(pos * div)
    return pe


def _build_nc(debug=False):
    import concourse.bacc as bacc
    import concourse.mybir as mybir
    from concourse.tile import TileContext

    dt = mybir.dt
    F32, BF16, I16 = dt.float32, dt.bfloat16, dt.int16
    AL = mybir.AluOpType
    AX = mybir.AxisListType

    nc = bacc.Bacc(None, target_bir_lowering=False)

    def din(name, shape, dtyp):
        return nc.dram_tensor(name, shape, dtyp, kind="ExternalInput")

    gt_d = din("gt", [128, GT], F32)
    th_d = din("th", [128, 1024], BF16)
    tlo_d = din("tlo", [128, 1024], BF16)
    idx_d = din("idx", [128, NI // 16], I16)
    lstmT_d = din("lstmT", [64, 2048], F32)
    firstT_d = din("firstT", [128, 8], F32)
    fstat_d = din("fstat", [128, 1], F32)
    wstat_d = din("wstat", [64, 2], F32)
    iota16_d = din("iota16", [128, 16], BF16)
    iota13_d = din("iota13", [128, 13], BF16)
    e127_d = din("e127", [128, 1], BF16)
    id128_d = din("id128", [128, 128], F32)
    sel8_d = din("sel8", [128, 8], F32)
    t8m_d = din("t8m", [8, 128], F32)
    ones128_d = din("ones128", [1, 128], F32)
    pec_d = din("pec", [128, 104], BF16)
    w1a_d = din("w1a", [128, 32], F32)
    w1b_d = din("w1b", [128, 32], F32)
    w2_d = din("w2", [33, 16], F32)
    wf1a_d = din("wf1a", [33, 32], F32)
    wf1b_d = din("wf1b", [33, 32], F32)
    wf2_d = din("wf2", [33, 16], F32)
    wt1_d = din("wt1", [33, 16], F32)
    wt2_d = din("wt2", [33, 1], F32)
    score_d = nc.dram_tensor("score", [1, 8], F32, kind="ExternalOutput")
    if debug:
        dbg = {n: nc.dram_tensor("dbg_" + n, shp, dty, kind="ExternalOutput")
               for n, shp, dty in [
                   ("G", [128, NI], F32), ("kcol", [128, 1], F32),
                   ("A", [128, 1], F32), ("astat", [128, 256], BF16),
                   ("PH", [128, 1024], BF16), ("hirows", [1, 1024], F32),
                   ("lorows", [1, 416], F32), ("h1", [128, 16], F32),
                   ("h2", [128, 16], F32), ("p1", [104, 16], F32),
                   ("p2", [64, 16], F32), ("gt", [128, GT], F32)]}

    with TileContext(nc) as tc:
        with tc.tile_pool(name="const", bufs=1) as cp, \
             tc.tile_pool(name="work", bufs=1) as wp:
            def load(dram, shape, dtyp):
                t = cp.tile(shape, dtyp, tag=dram.name)
                nc.sync.dma_start(t[:], dram[:])
                return t

            gt = load(gt_d, [128, GT], F32)
            th = load(th_d, [128, 1024], BF16)
            tlo = load(tlo_d, [128, 1024], BF16)
            idx = load(idx_d, [128, NI // 16], I16)
            lstmT = load(lstmT_d, [64, 2048], F32)
            firstT = load(firstT_d, [128, 8], F32)
            fstat = load(fstat_d, [128, 1], F32)
            wstat = load(wstat_d, [64, 2], F32)
            iota16 = load(iota16_d, [128, 16], BF16)
            iota13 = load(iota13_d, [128, 13], BF16)
            e127 = load(e127_d, [128, 1], BF16)
            id128 = load(id128_d, [128, 128], F32)
            sel8 = load(sel8_d, [128, 8], F32)
            t8m = load(t8m_d, [8, 128], F32)
            ones128 = load(ones128_d, [1, 128], F32)
            pec = load(pec_d, [128, 104], BF16)
            w1a = load(w1a_d, [128, 32], F32)
            w1b = load(w1b_d, [128, 32], F32)
            w2 = load(w2_d, [33, 16], F32)
            wf1a = load(wf1a_d, [33, 32], F32)
            wf1b = load(wf1b_d, [33, 32], F32)
            wf2 = load(wf2_d, [33, 16], F32)
            wt1 = load(wt1_d, [33, 16], F32)
            wt2 = load(wt2_d, [33, 1], F32)

            # ---- lo one-hot (DVE; one big fused instruction) ----
            # oh_lo: [p, jj, col(256), lo(13)]; tlo host layout [p, jj*256+col]
            oh_lo = wp.tile([128, 13312], BF16)
            ohl4 = oh_lo[:].rearrange("p (j c h) -> p j c h", j=4, h=13)
            nc.vector.tensor_tensor(
                ohl4,
                iota13[:].rearrange("p (j c h) -> p j c h", j=1, c=1)
                .broadcast_to((128, 4, 256, 13)),
                tlo[:].rearrange("p (j c h) -> p j c h", j=4, h=1)
                .broadcast_to((128, 4, 256, 13)),
                AL.is_equal)

            # ---- per-batch lstm tables into gather table (PE) ----
            # gt layout: [0:600) wn tables, [600:1112) Ln batch-pairs,
            # [1112:1712) wl tables, [1712:2224) Ll batch-pairs.
            lrowA = wp.tile([1, 2048], F32)
            lrowB = wp.tile([1, 2048], F32)
            with tc.tile_pool(name="psL", bufs=1, space="PSUM") as pL:
                for half, dst in ((0, lrowA), (1, lrowB)):
                    lr_ps = pL.tile([1, 2048], F32, tag="lr")
                    for c in range(4):
                        nc.tensor.matmul(lr_ps[:, c * 512:(c + 1) * 512],
                                         wstat[:, half:half + 1],
                                         lstmT[:, c * 512:(c + 1) * 512],
                                         start=True, stop=True)
                    nc.vector.tensor_copy(dst[:], lr_ps[:])
            with tc.tile_pool(name="psB", bufs=2, space="PSUM") as pB:
                for off, src in ((600, lrowA), (1712, lrowB)):
                    for pr in range(4):      # batch pair (2*pr, 2*pr+1)
                        bc = pB.tile([128, 512], F32, tag="bc")
                        nc.tensor.matmul(bc[:], ones128[:],
                                         src[0:1, pr * 512:(pr + 1) * 512],
                                         start=True, stop=True)
                        nc.vector.tensor_copy(
                            gt[32 * pr:32 * pr + 32, off:off + 512],
                            bc[32 * pr:32 * pr + 32, :])

            # ---- the big gather (GPSIMD) ----
            G = wp.tile([128, NI], F32)
            nc.gpsimd.ap_gather(G[:], gt[:], idx[:], channels=128,
                                num_elems=GT, d=1, num_idxs=NI)

            # ---- k_b (per-batch scalar, replicated on partitions) ----
            with tc.tile_pool(name="psK", bufs=1, space="PSUM") as pK:
                kf_ps = pK.tile([1, 8], F32, tag="kfa")
                nc.tensor.matmul(kf_ps[:], fstat[:], firstT[:],
                                 start=True, stop=True)
                kf_sb = wp.tile([1, 8], F32)
                nc.vector.tensor_copy(kf_sb[:], kf_ps[:])
                kfb_ps = pK.tile([8, 1], F32, tag="kfb")
                nc.tensor.transpose(kfb_ps[:], kf_sb[:], id128[0:1, 0:1])
                fk8 = wp.tile([8, 1], F32)
                nc.vector.tensor_copy(fk8[:], kfb_ps[:])
                kfc_ps = pK.tile([128, 1], F32, tag="kfc")
                nc.tensor.matmul(kfc_ps[:], t8m[:], fk8[:],
                                 start=True, stop=True)
                kg = wp.tile([128, 1], F32)
                nc.vector.tensor_reduce(kg[:], G[:, 16384:16388], AX.X, AL.add)
                kcol = wp.tile([128, 1], F32)
                nc.vector.tensor_tensor(kcol[:], kg[:], kfc_ps[:], AL.add)

            # ---- att = q0+q1+q2+q3+k into G[:,4096:8192]; A = row sums ----
            nc.vector.tensor_tensor(G[:, 0:4096], G[:, 0:4096],
                                    G[:, 4096:8192], AL.add)
            nc.vector.tensor_tensor(G[:, 8192:12288], G[:, 8192:12288],
                                    G[:, 12288:16384], AL.add)
            A_sb = wp.tile([128, 1], F32)
            nc.vector.scalar_tensor_tensor(G[:, 4096:8192], G[:, 0:4096],
                                           kcol[:], G[:, 8192:12288],
                                           AL.add, AL.add, accum_out=A_sb[:])

            # ---- att -> stationary layout att_stat[p, ch*8+b] ----
            att_stat = wp.tile([128, 256], BF16)
            with tc.tile_pool(name="psT", bufs=2, space="PSUM") as pT:
                for g in range(8):
                    tr = pT.tile([128, 512], F32, tag="tr")
                    for q in range(4):
                        ch = g * 4 + q
                        nc.tensor.transpose(
                            tr[:, q * 128:(q + 1) * 128],
                            G[:, 4096 + ch * 128:4096 + (ch + 1) * 128],
                            id128[:])
                    nc.vector.tensor_copy(
                        att_stat[:, g * 32:(g + 1) * 32]
                        .rearrange("p (q b r) -> p q b r", q=4, r=1),
                        tr[:].rearrange("p (q b r) -> p q b r",
                                        q=4, r=16)[:, :, :, 0:1])

            # ---- last-node one-hot rows via a mini hi one-hot (cols 248..255)
            hirows = wp.tile([1, 1024], F32)
            lorows = wp.tile([1, 416], F32)
            nc.vector.memset(hirows[:], 0.0)
            moh = wp.tile([128, 1024], BF16)    # [p, colm(8), jj(4), hi-pad(32)]
            vm = moh[:].rearrange("p (c j h) -> p c j h", c=8, j=4)
            nc.vector.memset(vm[:, :, :, 16:32], 0.0)
            nc.vector.tensor_tensor(
                vm[:, :, :, 0:16],
                iota16[:].rearrange("p (c j h) -> p c j h", c=1, j=1)
                .broadcast_to((128, 8, 4, 16)),
                th[:, 992:1024].rearrange("p (c j h) -> p c j h", j=4, h=1)
                .broadcast_to((128, 8, 4, 16)),
                AL.is_equal)
            with tc.tile_pool(name="psE", bufs=2, space="PSUM") as pE:
                for b in range(BPC):
                    ex = pE.tile([1, 64], F32, tag="ex")
                    nc.tensor.matmul(ex[:], e127[:], vm[:, b:b + 1, :, 0:16],
                                     start=True, stop=True)
                    nc.vector.tensor_copy(
                        hirows[0:1, b * 128:(b + 1) * 128]
                        .rearrange("p (a j h) -> p a j h",
                                   a=1, j=4)[:, :, :, 0:16],
                        ex[:].rearrange("p (a j h) -> p a j h", a=1, j=4))
                    ex2 = pE.tile([1, 52], F32, tag="ex2")
                    nc.tensor.matmul(ex2[:], e127[:],
                                     ohl4[:, :, 248 + b:249 + b, :],
                                     start=True, stop=True)
                    nc.vector.tensor_copy(lorows[0:1, b * 52:(b + 1) * 52],
                                          ex2[:])

            # ---- hi one-hots (tiled by col-group) + histograms ----
            # ohg: [p, colg(64), jj(4), hi-pad(32)]; th host [p, col*4+jj].
            # One PSUM bank per batch: a matmul with start=True marks its
            # whole 2KB bank pending-zero, so accumulation groups must not
            # share banks with other groups' starts.
            PH = wp.tile([128, 1024], BF16)
            with tc.tile_pool(name="psC", bufs=1, space="PSUM") as pC, \
                 tc.tile_pool(name="ohp", bufs=2) as ohp:
                pcs = [pC.tile([128, 52], F32, tag=f"pc{b}", name=f"pc{b}")
                       for b in range(BPC)]
                for g in range(4):
                    ohg = ohp.tile([128, 8192], BF16, tag="ohg")
                    v = ohg[:].rearrange("p (c j h) -> p c j h", c=64, j=4)
                    nc.vector.memset(v[:, :, :, 16:32], 0.0)
                    nc.vector.tensor_tensor(
                        v[:, :, :, 0:16],
                        iota16[:].rearrange("p (c j h) -> p c j h", c=1, j=1)
                        .broadcast_to((128, 64, 4, 16)),
                        th[:, g * 256:(g + 1) * 256]
                        .rearrange("p (c j h) -> p c j h", j=4, h=1)
                        .broadcast_to((128, 64, 4, 16)),
                        AL.is_equal)
                    nc.vector.tensor_tensor(
                        v[:, :, :, 0:16], v[:, :, :, 0:16],
                        att_stat[:, g * 64:(g + 1) * 64]
                        .rearrange("p (c j h) -> p c j h", j=1, h=1)
                        .broadcast_to((128, 64, 4, 16)),
                        AL.mult)
                    for b in range(BPC):
                        for chl in range(8):
                            ch = g * 8 + chl
                            colg = chl * 8 + b
                            nc.tensor.matmul(
                                pcs[b][:],
                                ohg[:, colg * 128:(colg + 1) * 128],
                                ohl4[:, :, ch * 8 + b:ch * 8 + b + 1, :],
                                start=(ch == 0), stop=(ch == NCH - 1))
                for b in range(BPC):
                    nc.vector.tensor_copy(PH[:, b * 52:(b + 1) * 52], pcs[b][:])
            with tc.tile_pool(name="psD", bufs=2, space="PSUM") as pD:
                for b in range(BPC):
                    po = pD.tile([128, 52], F32, tag="po")
                    nc.tensor.matmul(po[:],
                                     hirows[0:1, b * 128:(b + 1) * 128],
                                     lorows[0:1, b * 52:(b + 1) * 52],
                                     start=True, stop=True)
                    nc.vector.tensor_copy(PH[:, 512 + b * 52:512 + (b + 1) * 52],
                                          po[:])

            # ---- lstm projection tables (tiny PE transposes) ----
            lstm2dT = wp.tile([16, 832 * 8], BF16)  # per b: 13 x [16, 64]
            with tc.tile_pool(name="psM", bufs=2, space="PSUM") as pM:
                for b in range(BPC):
                    trm = pM.tile([16, 832], F32, tag="trm")
                    for g in range(13):
                        nc.tensor.transpose(
                            trm[:, g * 64:(g + 1) * 64],
                            lstmT[:, b * 256 + g * 16:b * 256 + (g + 1) * 16],
                            id128[0:64, 0:64])
                    nc.vector.tensor_copy(lstm2dT[:, b * 832:(b + 1) * 832],
                                          trm[:])

            # ---- projections -> hidden_in^T; MLP tail ----
            with tc.tile_pool(name="psP", bufs=1, space="PSUM") as pP:
                P1 = pP.tile([104, 16], F32, tag="p1")
                for jj in (1, 2, 3):
                    phr = PH[32 * jj:32 * jj + 16, :] \
                        .rearrange("p (s q) -> p s q", s=2)[:, :, 0:416] \
                        .rearrange("p s (b x) -> p s b x", b=8)
                    for lo in range(13):
                        nc.tensor.matmul(
                            P1[32 * jj:32 * jj + 8, :],
                            pec[32 * jj:32 * jj + 16, lo * 8:(lo + 1) * 8],
                            phr[:, :, :, jj * 13 + lo:jj * 13 + lo + 1],
                            start=(lo == 0), stop=(lo == 12),
                            tile_position=(32 * jj, 32 * jj))
                P2 = pP.tile([64, 16], F32, tag="p2")
                ph0 = PH[0:16, :].rearrange("p (s q) -> p s q", s=2)
                for b in range(BPC):
                    for lo in range(13):
                        nc.tensor.matmul(
                            P2[:, 2 * b:2 * b + 2],
                            lstm2dT[:, b * 832 + lo * 64:b * 832 + (lo + 1) * 64],
                            ph0[:, :, b * 52 + lo:b * 52 + lo + 1],
                            start=(lo == 0), stop=(lo == 12))

                Arow_ps = pP.tile([1, 8], F32, tag="ar")
                nc.tensor.matmul(Arow_ps[:], A_sb[:], sel8[:],
                                 start=True, stop=True)
                Arow = wp.tile([1, 8], F32)
                nc.vector.tensor_copy(Arow[:], Arow_ps[:])
                Abc_ps = pP.tile([128, 8], F32, tag="ab")
                nc.tensor.matmul(Abc_ps[64:128, :], ones128[0:1, 0:64],
                                 Arow[:], start=True, stop=True)

                # hidden_in^T in 32-aligned layout (W1 host-permuted):
                # hidden1: 0:32 lstm[0:32], 32:40 pe-pos, 64:72 pe-par,
                #          96:104 emb
                # hidden2: 0 ones, 32:64 lstm[32:64], 64:128 first
                hidden1 = wp.tile([128, 16], F32)
                hidden2 = wp.tile([128, 16], F32)
                nc.vector.memset(hidden1[:], 0.0)
                nc.vector.memset(hidden2[:], 0.0)
                nc.vector.memset(hidden2[0:1, :], 1.0)
                p1r = P1[:].rearrange("p (s b) -> p s b", s=2)
                for jj in (1, 2, 3):
                    nc.vector.tensor_copy(
                        hidden1[32 * jj:32 * jj + 8, :]
                        .rearrange("p (b s) -> p b s", s=2)[:, :, 1:2],
                        p1r[32 * jj:32 * jj + 8, 0:1, :]
                        .rearrange("p s b -> p b s"))
                    nc.vector.tensor_copy(
                        hidden1[32 * jj:32 * jj + 8, :]
                        .rearrange("p (b s) -> p b s", s=2)[:, :, 0:1],
                        p1r[32 * jj:32 * jj + 8, 1:2, :]
                        .rearrange("p s b -> p b s"))
                p2r = P2[:].rearrange("p (b s) -> p b s", s=2)
                nc.vector.tensor_copy(
                    hidden1[0:32, :].rearrange("p (b s) -> p b s", s=2)[:, :, 1:2],
                    p2r[0:32, :, 0:1])
                nc.vector.tensor_copy(
                    hidden1[0:32, :].rearrange("p (b s) -> p b s", s=2)[:, :, 0:1],
                    p2r[0:32, :, 1:2])
                nc.vector.tensor_copy(
                    hidden2[32:64, :].rearrange("p (b s) -> p b s", s=2)[:, :, 1:2],
                    p2r[32:64, :, 0:1])
                nc.vector.tensor_copy(
                    hidden2[32:64, :].rearrange("p (b s) -> p b s", s=2)[:, :, 0:1],
                    p2r[32:64, :, 1:2])
                nc.vector.tensor_copy(
                    hidden2[64:128, :].rearrange("p (b s) -> p b s", s=2)[:, :, 0:1],
                    firstT[64:128, :].rearrange("p (b s) -> p b s", s=1))
                nc.vector.tensor_tensor(
                    hidden2[64:128, :].rearrange("p (b s) -> p b s", s=2)[:, :, 1:2],
                    Abc_ps[64:128, :].rearrange("p (b s) -> p b s", s=1),
                    firstT[64:128, :].rearrange("p (b s) -> p b s", s=1),
                    AL.mult)

                ph1 = pP.tile([32, 16], F32, tag="mlp")
                nc.tensor.matmul(ph1[:], w1a[:], hidden1[:],
                                 start=True, stop=False)
                nc.tensor.matmul(ph1[:], w1b[:], hidden2[:],
                                 start=False, stop=True)
                h1s = wp.tile([33, 16], F32)
                nc.vector.tensor_relu(h1s[0:32, :], ph1[:])
                nc.vector.memset(h1s[32:33, :], 1.0)
                ph2 = pP.tile([16, 16], F32, tag="mlp")
                nc.tensor.matmul(ph2[:], w2[:], h1s[:], start=True, stop=True)
                h2s = wp.tile([33, 16], F32)
                nc.vector.memset(h2s[:], 0.0)
                nc.vector.memset(h2s[32:33, :], 1.0)
                nc.vector.tensor_relu(h2s[0:16, :], ph2[:])
                ps1 = pP.tile([32, 8], F32, tag="mlp")
                h2r = h2s[:].rearrange("p (b s) -> p b s", s=2)
                nc.tensor.matmul(ps1[:], wf1a[:], h2r[:, :, 0:1],
                                 start=True, stop=False)
                nc.tensor.matmul(ps1[:], wf1b[:], h2r[:, :, 1:2],
                                 start=False, stop=True)
                s1 = wp.tile([33, 8], F32)
                nc.vector.tensor_relu(s1[0:32, :], ps1[:])
                nc.vector.memset(s1[32:33, :], 1.0)
                ps2 = pP.tile([16, 8], F32, tag="mlp")
                nc.tensor.matmul(ps2[:], wf2[:], s1[:], start=True, stop=True)
                s2 = wp.tile([33, 8], F32)
                nc.vector.memset(s2[:], 0.0)
                nc.vector.memset(s2[32:33, :], 1.0)
                nc.vector.tensor_relu(s2[0:16, :], ps2[:])
                ps3 = pP.tile([16, 8], F32, tag="mlp")
                nc.tensor.matmul(ps3[:], wt1[:], s2[:], start=True, stop=True)
                s3 = wp.tile([33, 8], F32)
                nc.vector.memset(s3[:], 0.0)
                nc.vector.memset(s3[32:33, :], 1.0)
                nc.vector.tensor_copy(s3[0:16, :], ps3[:])
                psc = pP.tile([1, 8], F32, tag="mlp")
                nc.tensor.matmul(psc[:], wt2[:], s3[:], start=True, stop=True)
                score_sb = wp.tile([1, 8], F32)
                nc.vector.tensor_copy(score_sb[:], psc[:])
                nc.sync.dma_start(score_d[:], score_sb[:])

                if debug:
                    p1sb = wp.tile([104, 16], F32)
                    nc.vector.tensor_copy(p1sb[0:104, :], P1[:])
                    p2sb = wp.tile([64, 16], F32)
                    nc.vector.tensor_copy(p2sb[:], P2[:])
                    for name, src in [("G", G), ("kcol", kcol), ("A", A_sb),
                                      ("astat", att_stat), ("PH", PH),
                                      ("hirows", hirows), ("lorows", lorows),
                                      ("h1", hidden1), ("h2", hidden2),
                                      ("p1", p1sb), ("p2", p2sb), ("gt", gt)]:
                        nc.sync.dma_start(dbg[name][:], src[:])

    nc.finalize()
    return nc


def _host_prep(inputs):
    f32 = np.float32
    bf16 = ml_dtypes.bfloat16
    pe = _make_pe()
    emb = np.asarray(inputs["embedding"], f32)
    Wa1, Wa2 = np.asarray(inputs["Wa1"], f32), np.asarray(inputs["Wa2"], f32)
    ba1, ba2 = np.asarray(inputs["ba1"], f32), np.asarray(inputs["ba2"], f32)
    w = (Wa1 @ Wa2)[:, 0]
    c0 = float((ba1 @ Wa2 + ba2)[0])
    wl, wn = w[:152], w[152:]
    gt_static = np.zeros(GT, f32)
    gt_static[0:200] = pe @ wn[0:8]
    gt_static[200:400] = pe @ wn[8:16]
    gt_static[400:600] = emb @ wn[16:24]
    gt_static[1112:1312] = pe @ wl[0:8]
    gt_static[1312:1512] = pe @ wl[8:16]
    gt_static[1512:1712] = emb @ wl[16:24]
    gt_full = np.ascontiguousarray(np.broadcast_to(gt_static, (128, GT)))

    trees = np.asarray(inputs["trees"])
    lstm = np.asarray(inputs["lstm_out"], f32)
    first = np.asarray(inputs["first_notes"], f32)

    consts = {}
    fstat = np.zeros((128, 1), f32)
    fstat[0, 0] = c0
    fstat[64:128, 0] = wn[88:152] + wl[88:152]
    consts["fstat"] = fstat
    consts["wstat"] = np.stack([wn[24:88], wl[24:88]], 1).astype(f32)
    consts["iota16"] = np.ascontiguousarray(
        np.broadcast_to(np.arange(16, dtype=f32), (128, 16))).astype(bf16)
    consts["iota13"] = np.ascontiguousarray(
        np.broadcast_to(np.arange(13, dtype=f32), (128, 13))).astype(bf16)
    e127 = np.zeros((128, 1), f32); e127[127, 0] = 1.0
    consts["e127"] = e127.astype(bf16)
    consts["id128"] = np.eye(128, dtype=f32)
    sel8 = np.zeros((128, 8), f32)
    for b in range(8):
        sel8[16 * b, b] = 1.0
    consts["sel8"] = sel8
    t8m = np.zeros((8, 128), f32)
    for b in range(8):
        t8m[b, 16 * b:16 * b + 16] = 1.0
    consts["t8m"] = t8m
    consts["ones128"] = np.ones((1, 128), f32)
    pec = np.zeros((128, 104), f32)
    for jj, T in ((1, pe), (2, pe), (3, emb)):
        for lo in range(13):
            for hi in range(16):
                v = 16 * lo + hi
                if v < 200:
                    pec[32 * jj + hi, lo * 8:(lo + 1) * 8] = T[v]
    consts["pec"] = pec.astype(bf16)
    W1, b1 = np.asarray(inputs["W1"], f32), np.asarray(inputs["b1"], f32)
    w1a = np.zeros((128, 32), f32)
    w1a[0:32] = W1[24:56]       # lstm dims 0:32
    w1a[32:40] = W1[0:8]        # pe-pos
    w1a[64:72] = W1[8:16]       # pe-par
    w1a[96:104] = W1[16:24]     # emb
    consts["w1a"] = w1a
    w1b = np.zeros((128, 32), f32)
    w1b[0] = b1
    w1b[32:64] = W1[56:88]      # lstm dims 32:64
    w1b[64:128] = W1[88:152]    # first block
    consts["w1b"] = w1b
    consts["w2"] = np.vstack([np.asarray(inputs["W2"], f32),
                              np.asarray(inputs["b2"], f32)[None]])
    Wf1, bf1 = np.asarray(inputs["Wf1"], f32), np.asarray(inputs["bf1"], f32)
    consts["wf1a"] = np.vstack([Wf1[0:16], np.zeros((16, 32), f32), bf1[None]])
    consts["wf1b"] = np.vstack([Wf1[16:32], np.zeros((17, 32), f32)])
    consts["wf2"] = np.vstack([np.asarray(inputs["Wf2"], f32),
                               np.asarray(inputs["bf2"], f32)[None]])
    consts["wt1"] = np.vstack([np.asarray(inputs["Wt1"], f32),
                               np.zeros((16, 16), f32),
                               np.asarray(inputs["bt1"], f32)[None]])
    consts["wt2"] = np.vstack([np.asarray(inputs["Wt2"], f32),
                               np.zeros((16, 1), f32),
                               np.asarray(inputs["bt2"], f32)[None]])

    in_maps = []
    arr = trees.reshape(NCORES, BPC, NCH, 128, 4)    # [k, b, ch, p, j]
    for k in range(NCORES):
        m = dict(consts)
        m["gt"] = gt_full
        tj = arr[k].transpose(3, 0, 1, 2)            # [j, b, ch, p]
        th = np.empty((128, 1024), np.int32)
        tl = np.empty((128, 1024), np.int32)
        for jj, j in enumerate(JS):
            blk = tj[j].transpose(1, 0, 2).reshape(NCOL, 128).T  # [p, ch*8+b]
            th[:, jj::4] = blk % 16          # th is col-major: [p, col*4+jj]
            tl[:, jj * 256:(jj + 1) * 256] = blk // 16
        m["th"] = th.astype(f32).astype(bf16)
        m["tlo"] = tl.astype(f32).astype(bf16)
        IDX = np.zeros((BPC, NI), np.int16)
        t_k = arr[k].reshape(BPC, N, 4)
        half = np.arange(BPC) % 2                    # batch parity in pair
        IDX[:, 0:N] = t_k[:, :, 0]
        IDX[:, N:2 * N] = 200 + t_k[:, :, 1]
        IDX[:, 2 * N:3 * N] = 400 + t_k[:, :, 2]
        IDX[:, 3 * N:4 * N] = 600 + 256 * half[:, None] + t_k[:, :, 3]
        tl4 = t_k[:, 4095, :]
        IDX[:, 16384] = 1112 + tl4[:, 0]
        IDX[:, 16385] = 1312 + tl4[:, 1]
        IDX[:, 16386] = 1512 + tl4[:, 2]
        IDX[:, 16387] = 1712 + 256 * half + tl4[:, 3]
        m["idx"] = np.ascontiguousarray(
            IDX.reshape(BPC, NI // 16, 16).transpose(0, 2, 1)
            .reshape(128, NI // 16))
        P = np.zeros((BPC, 256, 64), f32)
        P[:, :200, :] = lstm[8 * k:8 * (k + 1), :200, :]
        m["lstmT"] = np.ascontiguousarray(P.transpose(2, 0, 1).reshape(64, 2048))
        ft = np.zeros((128, 8), f32)
        ft[0, :] = 1.0
        ft[64:128] = first[8 * k:8 * (k + 1)].T
        m["firstT"] = ft
        in_maps.append(m)
    return in_maps


def kernel(**inputs):
    from concourse.bass_utils import run_bass_kernel_spmd
    if "nc" not in _CACHE:
        _CACHE["nc"] = _build_nc()
    in_maps = _host_prep(inputs)
    res = run_bass_kernel_spmd(_CACHE["nc"], in_maps,
                               core_ids=list(range(NCORES)))
    _CACHE["last_res"] = res
    out = np.concatenate([np.asarray(r["score"]).reshape(8)
                          for r in res.results])
    return out.reshape(B, 1).astype(np.float32)


# revision 29
# speedup vs baseline: 1.0227x; 1.0227x over previous
"""Trainium2 Bass kernel for nn_CodeSynthesisModel (gnn_message_passing).

Data-parallel over 8 NeuronCores: B=64 sharded 8 ways (8 batches/core),
weights replicated. All O(B*N) compute runs on device via a Bass/Tile
kernel (one NEFF, SPMD on cores 0-7 through run_bass_kernel_spmd).

Math (exact, validated in numpy vs the reference):
  - The attention scorer is affine: w = Wa1@Wa2 collapses it to
      att[n] = q[n] + k_b,   q[n] = sum_j g_j[t_j[n]],
    where g_j are 200-entry scalar lookup tables (weight-projected pe /
    embedding / per-batch lstm rows). All per-node gathers (4 cols x 4096
    nodes x 8 batches) are ONE gpsimd ap_gather from per-partition
    tables with per-Q7-core index streams.
  - att_sum = sum_n att[n] * node_vec[n] decomposes into weighted
    histograms. With t = 16*lo + hi the histogram H2d[hi,lo] is a PE
    matmul (att*onehot_hi)^T @ onehot_lo per (batch, 128-node chunk),
    accumulated in PSUM; one-hots built on DVE in 2 big instructions.
  - att_sum block j = sum_lo H2d[:,lo]^T @ T_j[16*lo:16*lo+16, :] -- 13
    small PE matmuls per j. `last` (node 4095) uses the same projection
    path with a rank-1 one-hot outer product.
  - The tail MLPs are tiny PE matmuls with bias rows folded in; the
    hidden vector is assembled in a 32-aligned row layout (DVE partition
    bases must be multiples of 32) and W1 is host-permuted to match.

Layout: col = ch*8 + b (ch = 128-node chunk, b = local batch); node
n = ch*128 + p. One-hot j-blocks in order JS = [3,0,1,2] so the lstm
block sits at rows 0:16 and pe/pe/emb at rows 32/64/96.
"""

import numpy as np
import ml_dtypes

B, N, V = 64, 4096, 200
NCORES, BPC = 8, 8
NCH = 32
NCOL = 256
NI = 16400                  # gather indices (16384 + 4 + 12 pad)
GT = 2224                   # gather table entries
JS = [3, 0, 1, 2]
JHI = 4112                  # per-j extent in oh_hi (257*16, one pad col)

_CACHE = {}


def _make_pe():
    pos = np.arange(V, dtype=np.float32)[:, None]
    div = np.exp(np.arange(0, 8, 2, dtype=np.float32) * (-np.log(10000.0) / 8))
    pe = np.zeros((V, 8), dtype=np.float32)
    pe[:, 0::2] = np.sin(pos * div)
    pe[:, 1::2] = np.cos(pos * div)
    return pe


def _build_nc(debug=False):
    import concourse.bacc as bacc
    import concourse.mybir as mybir
    from concourse.tile import TileContext

    dt = mybir.dt
    F32, BF16, I16 = dt.float32, dt.bfloat16, dt.int16
    AL = mybir.AluOpType
    AX = mybir.AxisListType

    nc = bacc.Bacc(None, target_bir_lowering=False)

    def din(name, shape, dtyp):
        return nc.dram_tensor(name, shape, dtyp, kind="ExternalInput")

    gt_d = din("gt", [128, GT], F32)
    th_d = din("th", [128, 1024], BF16)
    tlo_d = din("tlo", [128, 1024], BF16)
    idx_ds = [din(f"idx{g}", [128, 257 if g == 0 else 256], I16)
              for g in range(4)]
    lstmT_d = din("lstmT", [64, 2048], F32)
    firstT_d = din("firstT", [128, 8], F32)
    fstat_d = din("fstat", [128, 1], F32)
    wstat_d = din("wstat", [64, 2], F32)
    iota16_d = din("iota16", [128, 16], BF16)
    iota13_d = din("iota13", [128, 13], BF16)
    e127_d = din("e127", [128, 1], BF16)
    id128_d = din("id128", [128, 128], F32)
    sel8_d = din("sel8", [128, 8], F32)
    t8m_d = din("t8m", [8, 128], F32)
    ones128_d = din("ones128", [1, 128], F32)
    pec_d = din("pec", [128, 104], BF16)
    w1a_d = din("w1a", [128, 32], F32)
    w1b_d = din("w1b", [128, 32], F32)
    w2_d = din("w2", [33, 16], F32)
    wf1a_d = din("wf1a", [33, 32], F32)
    wf1b_d = din("wf1b", [33, 32], F32)
    wf2_d = din("wf2", [33, 16], F32)
    wt1_d = din("wt1", [33, 16], F32)
    wt2_d = din("wt2", [33, 1], F32)
    score_d = nc.dram_tensor("score", [1, 8], F32, kind="ExternalOutput")
    if debug:
        dbg = {n: nc.dram_tensor("dbg_" + n, shp, dty, kind="ExternalOutput")
               for n, shp, dty in [
                   ("G", [128, 16528], F32), ("kcol", [128, 1], F32),
                   ("A", [128, 1], F32), ("astat", [128, 256], BF16),
                   ("PH", [128, 1024], BF16), ("hirows", [1, 1024], F32),
                   ("lorows", [1, 416], F32), ("h1", [128, 16], F32),
                   ("h2", [128, 16], F32), ("p1", [104, 16], F32),
                   ("p2", [64, 16], F32), ("gt", [128, GT], F32)]}

    with TileContext(nc) as tc:
        with tc.tile_pool(name="const", bufs=1) as cp, \
             tc.tile_pool(name="work", bufs=1) as wp:
            def load(dram, shape, dtyp):
                t = cp.tile(shape, dtyp, tag=dram.name)
                nc.sync.dma_start(t[:], dram[:])
                return t

            gt = load(gt_d, [128, GT], F32)
            lstmT = load(lstmT_d, [64, 2048], F32)
            wstat = load(wstat_d, [64, 2], F32)
            ones128 = load(ones128_d, [1, 128], F32)
            idxs = [load(idx_ds[g], [128, 257 if g == 0 else 256], I16)
                    for g in range(4)]
            th = load(th_d, [128, 1024], BF16)
            tlo = load(tlo_d, [128, 1024], BF16)
            firstT = load(firstT_d, [128, 8], F32)
            fstat = load(fstat_d, [128, 1], F32)
            iota16 = load(iota16_d, [128, 16], BF16)
            iota13 = load(iota13_d, [128, 13], BF16)
            e127 = load(e127_d, [128, 1], BF16)
            id128 = load(id128_d, [128, 128], F32)
            sel8 = load(sel8_d, [128, 8], F32)
            t8m = load(t8m_d, [8, 128], F32)
            pec = load(pec_d, [128, 104], BF16)
            w1a = load(w1a_d, [128, 32], F32)
            w1b = load(w1b_d, [128, 32], F32)
            w2 = load(w2_d, [33, 16], F32)
            wf1a = load(wf1a_d, [33, 32], F32)
            wf1b = load(wf1b_d, [33, 32], F32)
            wf2 = load(wf2_d, [33, 16], F32)
            wt1 = load(wt1_d, [33, 16], F32)
            wt2 = load(wt2_d, [33, 1], F32)

            # ---- lo one-hot (DVE; one big fused instruction) ----
            # oh_lo: [p, jj, col(256), lo(13)]; tlo host layout [p, jj*256+col]
            oh_lo = wp.tile([128, 13312], BF16)
            ohl4 = oh_lo[:].rearrange("p (j c h) -> p j c h", j=4, h=13)
            nc.vector.tensor_tensor(
                ohl4,
                iota13[:].rearrange("p (j c h) -> p j c h", j=1, c=1)
                .broadcast_to((128, 4, 256, 13)),
                tlo[:].rearrange("p (j c h) -> p j c h", j=4, h=1)
                .broadcast_to((128, 4, 256, 13)),
                AL.is_equal)

            # ---- per-batch lstm tables into gather table (PE) ----
            # gt layout: [0:600) wn tables, [600:1112) Ln batch-pairs,
            # [1112:1712) wl tables, [1712:2224) Ll batch-pairs.
            lrowA = wp.tile([1, 2048], F32)
            lrowB = wp.tile([1, 2048], F32)
            with tc.tile_pool(name="psL", bufs=1, space="PSUM") as pL:
                for half, dst in ((0, lrowA), (1, lrowB)):
                    lr_ps = pL.tile([1, 2048], F32, tag="lr")
                    for c in range(4):
                        nc.tensor.matmul(lr_ps[:, c * 512:(c + 1) * 512],
                                         wstat[:, half:half + 1],
                                         lstmT[:, c * 512:(c + 1) * 512],
                                         start=True, stop=True)
                    nc.vector.tensor_copy(dst[:], lr_ps[:])
            with tc.tile_pool(name="psB", bufs=2, space="PSUM") as pB:
                for off, src in ((600, lrowA), (1712, lrowB)):
                    for pr in range(4):      # batch pair (2*pr, 2*pr+1)
                        bc = pB.tile([128, 512], F32, tag="bc")
                        nc.tensor.matmul(bc[:], ones128[:],
                                         src[0:1, pr * 512:(pr + 1) * 512],
                                         start=True, stop=True)
                        nc.vector.tensor_copy(
                            gt[32 * pr:32 * pr + 32, off:off + 512],
                            bc[32 * pr:32 * pr + 32, :])

            # ---- gathers (GPSIMD), 4 node-range chunks for pipelining ----
            # G block g at off(g): [j0|j1|j2|j3] x 1024 nodes (+4 k-idx +12
            # pad in block 0). att_g overwrites the j1 slot of each block.
            G = wp.tile([128, 16528], F32)
            offs = [0, 4112, 8208, 12304]
            for g in range(4):
                n = 4112 if g == 0 else 4096
                nc.gpsimd.ap_gather(G[:, offs[g]:offs[g] + n], gt[:],
                                    idxs[g][:], channels=128,
                                    num_elems=GT, d=1, num_idxs=n)

            # ---- k_b (per-batch scalar, replicated on partitions) ----
            with tc.tile_pool(name="psK", bufs=1, space="PSUM") as pK:
                kf_ps = pK.tile([1, 8], F32, tag="kfa")
                nc.tensor.matmul(kf_ps[:], fstat[:], firstT[:],
                                 start=True, stop=True)
                kf_sb = wp.tile([1, 8], F32)
                nc.vector.tensor_copy(kf_sb[:], kf_ps[:])
                kfb_ps = pK.tile([8, 1], F32, tag="kfb")
                nc.tensor.transpose(kfb_ps[:], kf_sb[:], id128[0:1, 0:1])
                fk8 = wp.tile([8, 1], F32)
                nc.vector.tensor_copy(fk8[:], kfb_ps[:])
                kfc_ps = pK.tile([128, 1], F32, tag="kfc")
                nc.tensor.matmul(kfc_ps[:], t8m[:], fk8[:],
                                 start=True, stop=True)
                kg = wp.tile([128, 1], F32)
                nc.vector.tensor_reduce(kg[:], G[:, 4096:4100], AX.X, AL.add)
                kcol = wp.tile([128, 1], F32)
                nc.vector.tensor_tensor(kcol[:], kg[:], kfc_ps[:], AL.add)

            # ---- per chunk: att into j1 slot; transpose to att_stat ----
            att_stat = wp.tile([128, 256], BF16)
            Ags = []
            with tc.tile_pool(name="psT", bufs=2, space="PSUM") as pT:
                for g in range(4):
                    base = offs[g]
                    nc.vector.tensor_tensor(
                        G[:, base:base + 1024], G[:, base:base + 1024],
                        G[:, base + 1024:base + 2048], AL.add)
                    nc.vector.tensor_tensor(
                        G[:, base + 2048:base + 3072],
                        G[:, base + 2048:base + 3072],
                        G[:, base + 3072:base + 4096], AL.add)
                    ag = wp.tile([128, 1], F32, name=f"ag{g}", tag=f"ag{g}")
                    Ags.append(ag)
                    nc.vector.scalar_tensor_tensor(
                        G[:, base + 1024:base + 2048], G[:, base:base + 1024],
                        kcol[:], G[:, base + 2048:base + 3072],
                        AL.add, AL.add, accum_out=ag[:])
                    for half in range(2):
                        tr = pT.tile([128, 512], F32, tag="tr")
                        for q in range(4):
                            o = base + 1024 + (half * 4 + q) * 128
                            nc.tensor.transpose(tr[:, q * 128:(q + 1) * 128],
                                                G[:, o:o + 128], id128[:])
                        nc.vector.tensor_copy(
                            att_stat[:, g * 64 + half * 32:
                                     g * 64 + (half + 1) * 32]
                            .rearrange("p (q b r) -> p q b r", q=4, r=1),
                            tr[:].rearrange("p (q b r) -> p q b r",
                                            q=4, r=16)[:, :, :, 0:1])
            A_sb = wp.tile([128, 1], F32)
            nc.vector.tensor_tensor(Ags[0][:], Ags[0][:], Ags[1][:], AL.add)
            nc.vector.tensor_tensor(Ags[2][:], Ags[2][:], Ags[3][:], AL.add)
            nc.vector.tensor_tensor(A_sb[:], Ags[0][:], Ags[2][:], AL.add)

            # ---- last-node one-hot rows via a mini hi one-hot (cols 248..255)
            hirows = wp.tile([1, 1024], F32)
            lorows = wp.tile([1, 416], F32)
            nc.vector.memset(hirows[:], 0.0)
            moh = wp.tile([128, 1024], BF16)    # [p, colm(8), jj(4), hi-pad(32)]
            vm = moh[:].rearrange("p (c j h) -> p c j h", c=8, j=4)
            nc.vector.memset(vm[:, :, :, 16:32], 0.0)
            nc.vector.tensor_tensor(
                vm[:, :, :, 0:16],
                iota16[:].rearrange("p (c j h) -> p c j h", c=1, j=1)
                .broadcast_to((128, 8, 4, 16)),
                th[:, 992:1024].rearrange("p (c j h) -> p c j h", j=4, h=1)
                .broadcast_to((128, 8, 4, 16)),
                AL.is_equal)
            with tc.tile_pool(name="psE", bufs=2, space="PSUM") as pE:
                for b in range(BPC):
                    ex = pE.tile([1, 64], F32, tag="ex")
                    nc.tensor.matmul(ex[:], e127[:], vm[:, b:b + 1, :, 0:16],
                                     start=True, stop=True)
                    nc.vector.tensor_copy(
                        hirows[0:1, b * 128:(b + 1) * 128]
                        .rearrange("p (a j h) -> p a j h",
                                   a=1, j=4)[:, :, :, 0:16],
                        ex[:].rearrange("p (a j h) -> p a j h", a=1, j=4))
                    ex2 = pE.tile([1, 52], F32, tag="ex2")
                    nc.tensor.matmul(ex2[:], e127[:],
                                     ohl4[:, :, 248 + b:249 + b, :],
                                     start=True, stop=True)
                    nc.vector.tensor_copy(lorows[0:1, b * 52:(b + 1) * 52],
                                          ex2[:])

            # ---- hi one-hots (tiled by col-group) + histograms ----
            # ohg: [p, colg(64), jj(4), hi-pad(32)]; th host [p, col*4+jj].
            # One PSUM bank per batch: a matmul with start=True marks its
            # whole 2KB bank pending-zero, so accumulation groups must not
            # share banks with other groups' starts.
            PH = wp.tile([128, 1024], BF16)
            with tc.tile_pool(name="psC", bufs=1, space="PSUM") as pC, \
                 tc.tile_pool(name="ohp", bufs=2) as ohp:
                pcs = [pC.tile([128, 52], F32, tag=f"pc{b}", name=f"pc{b}")
                       for b in range(BPC)]
                for g in range(4):
                    ohg = ohp.tile([128, 8192], BF16, tag="ohg")
                    v = ohg[:].rearrange("p (c j h) -> p c j h", c=64, j=4)
                    nc.vector.memset(v[:, :, :, 16:32], 0.0)
                    nc.vector.tensor_tensor(
                        v[:, :, :, 0:16],
                        iota16[:].rearrange("p (c j h) -> p c j h", c=1, j=1)
                        .broadcast_to((128, 64, 4, 16)),
                        th[:, g * 256:(g + 1) * 256]
                        .rearrange("p (c j h) -> p c j h", j=4, h=1)
                        .broadcast_to((128, 64, 4, 16)),
                        AL.is_equal)
                    nc.vector.tensor_tensor(
                        v[:, :, :, 0:16], v[:, :, :, 0:16],
                        att_stat[:, g * 64:(g + 1) * 64]
                        .rearrange("p (c j h) -> p c j h", j=1, h=1)
                        .broadcast_to((128, 64, 4, 16)),
                        AL.mult)
                    for b in range(BPC):
                        for chl in range(8):
                            ch = g * 8 + chl
                            colg = chl * 8 + b
                            nc.tensor.matmul(
                                pcs[b][:],
                                ohg[:, colg * 128:(colg + 1) * 128],
                                ohl4[:, :, ch * 8 + b:ch * 8 + b + 1, :],
                                start=(ch == 0), stop=(ch == NCH - 1))
                for b in range(BPC):
                    nc.vector.tensor_copy(PH[:, b * 52:(b + 1) * 52], pcs[b][:])
            with tc.tile_pool(name="psD", bufs=2, space="PSUM") as pD:
                for b in range(BPC):
                    po = pD.tile([128, 52], F32, tag="po")
                    nc.tensor.matmul(po[:],
                                     hirows[0:1, b * 128:(b + 1) * 128],
                                     lorows[0:1, b * 52:(b + 1) * 52],
                                     start=True, stop=True)
                    nc.vector.tensor_copy(PH[:, 512 + b * 52:512 + (b + 1) * 52],
                                          po[:])

            # ---- lstm projection tables (tiny PE transposes) ----
            lstm2dT = wp.tile([16, 832 * 8], BF16)  # per b: 13 x [16, 64]
            with tc.tile_pool(name="psM", bufs=2, space="PSUM") as pM:
                for b in range(BPC):
                    trm = pM.tile([16, 832], F32, tag="trm")
                    for g in range(13):
                        nc.tensor.transpose(
                            trm[:, g * 64:(g + 1) * 64],
                            lstmT[:, b * 256 + g * 16:b * 256 + (g + 1) * 16],
                            id128[0:64, 0:64])
                    nc.vector.tensor_copy(lstm2dT[:, b * 832:(b + 1) * 832],
                                          trm[:])

            # ---- projections -> hidden_in^T; MLP tail ----
            with tc.tile_pool(name="psP", bufs=1, space="PSUM") as pP:
                P1 = pP.tile([104, 16], F32, tag="p1")
                for jj in (1, 2, 3):
                    phr = PH[32 * jj:32 * jj + 16, :] \
                        .rearrange("p (s q) -> p s q", s=2)[:, :, 0:416] \
                        .rearrange("p s (b x) -> p s b x", b=8)
                    for lo in range(13):
                        nc.tensor.matmul(
                            P1[32 * jj:32 * jj + 8, :],
                            pec[32 * jj:32 * jj + 16, lo * 8:(lo + 1) * 8],
                            phr[:, :, :, jj * 13 + lo:jj * 13 + lo + 1],
                            start=(lo == 0), stop=(lo == 12),
                            tile_position=(32 * jj, 32 * jj))
                P2 = pP.tile([64, 16], F32, tag="p2")
                ph0 = PH[0:16, :].rearrange("p (s q) -> p s q", s=2)
                for b in range(BPC):
                    for lo in range(13):
                        nc.tensor.matmul(
                            P2[:, 2 * b:2 * b + 2],
                            lstm2dT[:, b * 832 + lo * 64:b * 832 + (lo + 1) * 64],
                            ph0[:, :, b * 52 + lo:b * 52 + lo + 1],
                            start=(lo == 0), stop=(lo == 12))

                Arow_ps = pP.tile([1, 8], F32, tag="ar")
                nc.tensor.matmul(Arow_ps[:], A_sb[:], sel8[:],
                                 start=True, stop=True)
                Arow = wp.tile([1, 8], F32)
                nc.vector.tensor_copy(Arow[:], Arow_ps[:])
                Abc_ps = pP.tile([128, 8], F32, tag="ab")
                nc.tensor.matmul(Abc_ps[64:128, :], ones128[0:1, 0:64],
                                 Arow[:], start=True, stop=True)

                # hidden_in^T in 32-aligned layout (W1 host-permuted):
                # hidden1: 0:32 lstm[0:32], 32:40 pe-pos, 64:72 pe-par,
                #          96:104 emb
                # hidden2: 0 ones, 32:64 lstm[32:64], 64:128 first
                hidden1 = wp.tile([128, 16], F32)
                hidden2 = wp.tile([128, 16], F32)
                nc.vector.memset(hidden1[:], 0.0)
                nc.vector.memset(hidden2[:], 0.0)
                nc.vector.memset(hidden2[0:1, :], 1.0)
                p1r = P1[:].rearrange("p (s b) -> p s b", s=2)
                for jj in (1, 2, 3):
                    nc.vector.tensor_copy(
                        hidden1[32 * jj:32 * jj + 8, :]
                        .rearrange("p (b s) -> p b s", s=2)[:, :, 1:2],
                        p1r[32 * jj:32 * jj + 8, 0:1, :]
                        .rearrange("p s b -> p b s"))
                    nc.vector.tensor_copy(
                        hidden1[32 * jj:32 * jj + 8, :]
                        .rearrange("p (b s) -> p b s", s=2)[:, :, 0:1],
                        p1r[32 * jj:32 * jj + 8, 1:2, :]
                        .rearrange("p s b -> p b s"))
                p2r = P2[:].rearrange("p (b s) -> p b s", s=2)
                nc.vector.tensor_copy(
                    hidden1[0:32, :].rearrange("p (b s) -> p b s", s=2)[:, :, 1:2],
                    p2r[0:32, :, 0:1])
                nc.vector.tensor_copy(
                    hidden1[0:32, :].rearrange("p (b s) -> p b s", s=2)[:, :, 0:1],
                    p2r[0:32, :, 1:2])
                nc.vector.tensor_copy(
                    hidden2[32:64, :].rearrange("p (b s) -> p b s", s=2)[:, :, 1:2],
                    p2r[32:64, :, 0:1])
                nc.vector.tensor_copy(
                    hidden2[32:64, :].rearrange("p (b s) -> p b s", s=2)[:, :, 0:1],
                    p2r[32:64, :, 1:2])
                nc.vector.tensor_copy(
                    hidden2[64:128, :].rearrange("p (b s) -> p b s", s=2)[:, :, 0:1],
                    firstT[64:128, :].rearrange("p (b s) -> p b s", s=1))
                nc.vector.tensor_tensor(
                    hidden2[64:128, :].rearrange("p (b s) -> p b s", s=2)[:, :, 1:2],
                    Abc_ps[64:128, :].rearrange("p (b s) -> p b s", s=1),
                    firstT[64:128, :].rearrange("p (b s) -> p b s", s=1),
                    AL.mult)

                ph1 = pP.tile([32, 16], F32, tag="mlp")
                nc.tensor.matmul(ph1[:], w1a[:], hidden1[:],
                                 start=True, stop=False)
                nc.tensor.matmul(ph1[:], w1b[:], hidden2[:],
                                 start=False, stop=True)
                h1s = wp.tile([33, 16], F32)
                nc.vector.tensor_relu(h1s[0:32, :], ph1[:])
                nc.vector.memset(h1s[32:33, :], 1.0)
                ph2 = pP.tile([16, 16], F32, tag="mlp")
                nc.tensor.matmul(ph2[:], w2[:], h1s[:], start=True, stop=True)
                h2s = wp.tile([33, 16], F32)
                nc.vector.memset(h2s[:], 0.0)
                nc.vector.memset(h2s[32:33, :], 1.0)
                nc.vector.tensor_relu(h2s[0:16, :], ph2[:])
                ps1 = pP.tile([32, 8], F32, tag="mlp")
                h2r = h2s[:].rearrange("p (b s) -> p b s", s=2)
                nc.tensor.matmul(ps1[:], wf1a[:], h2r[:, :, 0:1],
                                 start=True, stop=False)
                nc.tensor.matmul(ps1[:], wf1b[:], h2r[:, :, 1:2],
                                 start=False, stop=True)
                s1 = wp.tile([33, 8], F32)
                nc.vector.tensor_relu(s1[0:32, :], ps1[:])
                nc.vector.memset(s1[32:33, :], 1.0)
                ps2 = pP.tile([16, 8], F32, tag="mlp")
                nc.tensor.matmul(ps2[:], wf2[:], s1[:], start=True, stop=True)
                s2 = wp.tile([33, 8], F32)
                nc.vector.memset(s2[:], 0.0)
                nc.vector.memset(s2[32:33, :], 1.0)
                nc.vector.tensor_relu(s2[0:16, :], ps2[:])
                ps3 = pP.tile([16, 8], F32, tag="mlp")
                nc.tensor.matmul(ps3[:], wt1[:], s2[:], start=True, stop=True)
                s3 = wp.tile([33, 8], F32)
                nc.vector.memset(s3[:], 0.0)
                nc.vector.memset(s3[32:33, :], 1.0)
                nc.vector.tensor_copy(s3[0:16, :], ps3[:])
                psc = pP.tile([1, 8], F32, tag="mlp")
                nc.tensor.matmul(psc[:], wt2[:], s3[:], start=True, stop=True)
                score_sb = wp.tile([1, 8], F32)
                nc.vector.tensor_copy(score_sb[:], psc[:])
                nc.sync.dma_start(score_d[:], score_sb[:])

                if debug:
                    p1sb = wp.tile([104, 16], F32)
                    nc.vector.tensor_copy(p1sb[0:104, :], P1[:])
                    p2sb = wp.tile([64, 16], F32)
                    nc.vector.tensor_copy(p2sb[:], P2[:])
                    for name, src in [("G", G), ("kcol", kcol), ("A", A_sb),
                                      ("astat", att_stat), ("PH", PH),
                                      ("hirows", hirows), ("lorows", lorows),
                                      ("h1", hidden1), ("h2", hidden2),
                                      ("p1", p1sb), ("p2", p2sb), ("gt", gt)]:
                        nc.sync.dma_start(dbg[name][:], src[:])

    nc.finalize()
    return nc


def _host_prep(inputs):
    f32 = np.float32
    bf16 = ml_dtypes.bfloat16
    pe = _make_pe()
    emb = np.asarray(inputs["embedding"], f32)
    Wa1, Wa2 = np.asarray(inputs["Wa1"], f32), np.asarray(inputs["Wa2"], f32)
    ba1, ba2 = np.asarray(inputs["ba1"], f32), np.asarray(inputs["ba2"], f32)
    w = (Wa1 @ Wa2)[:, 0]
    c0 = float((ba1 @ Wa2 + ba2)[0])
    wl, wn = w[:152], w[152:]
    gt_static = np.zeros(GT, f32)
    gt_static[0:200] = pe @ wn[0:8]
    gt_static[200:400] = pe @ wn[8:16]
    gt_static[400:600] = emb @ wn[16:24]
    gt_static[1112:1312] = pe @ wl[0:8]
    gt_static[1312:1512] = pe @ wl[8:16]
    gt_static[1512:1712] = emb @ wl[16:24]
    gt_full = np.ascontiguousarray(np.broadcast_to(gt_static, (128, GT)))

    trees = np.asarray(inputs["trees"])
    lstm = np.asarray(inputs["lstm_out"], f32)
    first = np.asarray(inputs["first_notes"], f32)

    consts = {}
    fstat = np.zeros((128, 1), f32)
    fstat[0, 0] = c0
    fstat[64:128, 0] = wn[88:152] + wl[88:152]
    consts["fstat"] = fstat
    consts["wstat"] = np.stack([wn[24:88], wl[24:88]], 1).astype(f32)
    consts["iota16"] = np.ascontiguousarray(
        np.broadcast_to(np.arange(16, dtype=f32), (128, 16))).astype(bf16)
    consts["iota13"] = np.ascontiguousarray(
        np.broadcast_to(np.arange(13, dtype=f32), (128, 13))).astype(bf16)
    e127 = np.zeros((128, 1), f32); e127[127, 0] = 1.0
    consts["e127"] = e127.astype(bf16)
    consts["id128"] = np.eye(128, dtype=f32)
    sel8 = np.zeros((128, 8), f32)
    for b in range(8):
        sel8[16 * b, b] = 1.0
    consts["sel8"] = sel8
    t8m = np.zeros((8, 128), f32)
    for b in range(8):
        t8m[b, 16 * b:16 * b + 16] = 1.0
    consts["t8m"] = t8m
    consts["ones128"] = np.ones((1, 128), f32)
    pec = np.zeros((128, 104), f32)
    for jj, T in ((1, pe), (2, pe), (3, emb)):
        for lo in range(13):
            for hi in range(16):
                v = 16 * lo + hi
                if v < 200:
                    pec[32 * jj + hi, lo * 8:(lo + 1) * 8] = T[v]
    consts["pec"] = pec.astype(bf16)
    W1, b1 = np.asarray(inputs["W1"], f32), np.asarray(inputs["b1"], f32)
    w1a = np.zeros((128, 32), f32)
    w1a[0:32] = W1[24:56]       # lstm dims 0:32
    w1a[32:40] = W1[0:8]        # pe-pos
    w1a[64:72] = W1[8:16]       # pe-par
    w1a[96:104] = W1[16:24]     # emb
    consts["w1a"] = w1a
    w1b = np.zeros((128, 32), f32)
    w1b[0] = b1
    w1b[32:64] = W1[56:88]      # lstm dims 32:64
    w1b[64:128] = W1[88:152]    # first block
    consts["w1b"] = w1b
    consts["w2"] = np.vstack([np.asarray(inputs["W2"], f32),
                              np.asarray(inputs["b2"], f32)[None]])
    Wf1, bf1 = np.asarray(inputs["Wf1"], f32), np.asarray(inputs["bf1"], f32)
    consts["wf1a"] = np.vstack([Wf1[0:16], np.zeros((16, 32), f32), bf1[None]])
    consts["wf1b"] = np.vstack([Wf1[16:32], np.zeros((17, 32), f32)])
    consts["wf2"] = np.vstack([np.asarray(inputs["Wf2"], f32),
                               np.asarray(inputs["bf2"], f32)[None]])
    consts["wt1"] = np.vstack([np.asarray(inputs["Wt1"], f32),
                               np.zeros((16, 16), f32),
                               np.asarray(inputs["bt1"], f32)[None]])
    consts["wt2"] = np.vstack([np.asarray(inputs["Wt2"], f32),
                               np.zeros((16, 1), f32),
                               np.asarray(inputs["bt2"], f32)[None]])

    in_maps = []
    arr = trees.reshape(NCORES, BPC, NCH, 128, 4)    # [k, b, ch, p, j]
    for k in range(NCORES):
        m = dict(consts)
        m["gt"] = gt_full
        tj = arr[k].transpose(3, 0, 1, 2)            # [j, b, ch, p]
        th = np.empty((128, 1024), np.int32)
        tl = np.empty((128, 1024), np.int32)
        for jj, j in enumerate(JS):
            blk = tj[j].transpose(1, 0, 2).reshape(NCOL, 128).T  # [p, ch*8+b]
            th[:, jj::4] = blk % 16          # th is col-major: [p, col*4+jj]
            tl[:, jj * 256:(jj + 1) * 256] = blk // 16
        m["th"] = th.astype(f32).astype(bf16)
        m["tlo"] = tl.astype(f32).astype(bf16)
        t_k = arr[k].reshape(BPC, N, 4)
        half = np.arange(BPC) % 2                    # batch parity in pair
        for g in range(4):
            n = 4112 if g == 0 else 4096
            IDX = np.zeros((BPC, n), np.int16)
            sl = slice(1024 * g, 1024 * (g + 1))
            IDX[:, 0:1024] = t_k[:, sl, 0]
            IDX[:, 1024:2048] = 200 + t_k[:, sl, 1]
            IDX[:, 2048:3072] = 400 + t_k[:, sl, 2]
            IDX[:, 3072:4096] = 600 + 256 * half[:, None] + t_k[:, sl, 3]
            if g == 0:
                tl4 = t_k[:, 4095, :]
                IDX[:, 4096] = 1112 + tl4[:, 0]
                IDX[:, 4097] = 1312 + tl4[:, 1]
                IDX[:, 4098] = 1512 + tl4[:, 2]
                IDX[:, 4099] = 1712 + 256 * half + tl4[:, 3]
            m[f"idx{g}"] = np.ascontiguousarray(
                IDX.reshape(BPC, n // 16, 16).transpose(0, 2, 1)
                .reshape(128, n // 16))
        P = np.zeros((BPC, 256, 64), f32)
        P[:, :200, :] = lstm[8 * k:8 * (k + 1), :200, :]
        m["lstmT"] = np.ascontiguousarray(P.transpose(2, 0, 1).reshape(64, 2048))
        ft = np.zeros((128, 8), f32)
        ft[0, :] = 1.0
        ft[64:128] = first[8 * k:8 * (k + 1)].T
        m["firstT"] = ft
        in_maps.append(m)
    return in_maps


def kernel(**inputs):
    from concourse.bass_utils import run_bass_kernel_spmd
    if "nc" not in _CACHE:
        _CACHE["nc"] = _build_nc()
    in_maps = _host_prep(inputs)
    res = run_bass_kernel_spmd(_CACHE["nc"], in_maps,
                               core_ids=list(range(NCORES)))
    _CACHE["last_res"] = res
    out = np.concatenate([np.asarray(r["score"]).reshape(8)
                          for r in res.results])
    return out.reshape(B, 1).astype(np.float32)
